# revision 19
# baseline (speedup 1.0000x reference)
"""Trainium2 Bass kernel for nn_Attention_10582799417937.

Data-parallel over batch (32 -> 4 per core x 8 cores), weights replicated.
Per-core pipeline (per batch):
  depthwise 3x3 convs (q-path on DVE/ACT, kv-path on gpsimd)
  -> pointwise projections (PE matmuls, bf16)
  -> attention computed transposed: dotsT[j,i] = k_h^T . q_h^T so the
     attention matrix never needs transposing; softmax denominators via
     ones-mask matmuls on PE (replicated across partitions), normalize on DVE.
  -> output projection (PE) -> DRAM.
All BN affine and the V-path bias are folded into weights on the host.
"""
import sys
import numpy as np
import ml_dtypes

sys.path.insert(0, "/opt/trn_rl_repo")

import concourse.bass as bass
import concourse.mybir as mybir
import concourse.tile as tile
from concourse import bacc
from concourse.bass_utils import run_bass_kernel_spmd

# ---- problem constants (hardcoded per spec) ----
B, C, H, W = 32, 384, 32, 32
HEADS, D = 6, 64
INNER = HEADS * D          # 384
SCALE = D ** -0.5
EPS = 1e-5
N_CORES = 8
B_LOC = B // N_CORES       # 4
HW = H * W                 # 1024
HK, WK = H // 2, W // 2
JK = HK * WK               # 256
KC = C // 128              # 3 channel chunks
MC = INNER // 128          # 3 inner chunks (also head pairs)
NPAIR = HEADS // 2         # 3

BF16 = mybir.dt.bfloat16
F32 = mybir.dt.float32
AL = mybir.AluOpType
AF = mybir.ActivationFunctionType

# tuning knobs
ACT_MUL_TAPS = {(0, 0), (0, 2), (2, 1)}  # q-conv taps whose multiply runs on ACT

# padded layout: [128, 34 rows x 34 cols]; data at rows 1..32, cols 1..32.
# Every tap reads a full [32, 32] window at offset (dy, dx) -> all ops
# full-range (partial-range DVE ops measured ~1.5x slower). Two batches are
# packed per tile ([128, 2, 34, 34]) to amortize per-op overhead.
PADR = 34
PADN = PADR * PADR

# tap order: center tap first (initializes accumulator)
ORDER = [(1, 1), (1, 0), (1, 2), (0, 0), (0, 1), (0, 2), (2, 0), (2, 1), (2, 2)]


GP_PAIR = ((2, 0), (2, 2))  # tap pair summed on gpsimd, joined by one DVE add


def _mul(nc, out, in_view, scal, on_act):
    if on_act:
        nc.scalar.mul(out, in_view, scal)
    else:
        nc.vector.tensor_scalar_mul(out, in_view, scal)


def _conv_s1(nc, scratch, xpv, taps_sb, acc):
    """Stride-1 3x3 depthwise conv over a 2-batch packed tile.
    Multiplies run per-batch (2-level APs keep the fast DVE mode); the
    accumulate adds run on the flat 2-batch [128, 2048] views.
    xpv: [128, 2, 34, 34]; acc: [128, 2048] bf16."""
    def win(h, dy, dx):
        return xpv[:, h, dy:dy + 32, dx:dx + W]

    gp_tmp = None
    for idx, (dy, dx) in enumerate(ORDER):
        t = dy * 3 + dx
        scal = taps_sb[:, t:t + 1]
        on_act = (dy, dx) in ACT_MUL_TAPS
        if idx == 0:
            for h in range(2):
                _mul(nc, acc[:, h * HW:(h + 1) * HW], win(h, dy, dx), scal, on_act)
            continue
        tmp = scratch.tile([128, 2 * HW], BF16, tag="cscr", name="cscr")
        for h in range(2):
            _mul(nc, tmp[:, h * HW:(h + 1) * HW], win(h, dy, dx), scal, on_act)
        if (dy, dx) in GP_PAIR:
            if gp_tmp is None:
                gp_tmp = tmp
            else:
                nc.gpsimd.tensor_tensor(gp_tmp[:], gp_tmp[:], tmp[:], AL.add)
                nc.vector.tensor_tensor(acc, acc, gp_tmp[:], AL.add)
        else:
            nc.vector.tensor_tensor(acc, acc, tmp[:], AL.add)


def _conv_s2(nc, scratch, xpv, taps_sb, acc):
    """Stride-2 3x3 depthwise conv, 2-batch packed. DVE multiplies (per
    batch) feed a serial gpsimd accumulate chain. acc: [128, 512] bf16."""
    for idx, (dy, dx) in enumerate(ORDER):
        t = dy * 3 + dx
        scal = taps_sb[:, t:t + 1]
        if idx == 0:
            for h in range(2):
                nc.vector.tensor_scalar_mul(
                    acc[:, h * JK:(h + 1) * JK],
                    xpv[:, h, dy:dy + 31:2, dx:dx + 31:2], scal)
            continue
        tmp = scratch.tile([128, 2 * JK], BF16, tag="kscr", name="kscr")
        for h in range(2):
            nc.vector.tensor_scalar_mul(
                tmp[:, h * JK:(h + 1) * JK],
                xpv[:, h, dy:dy + 31:2, dx:dx + 31:2], scal)
        nc.gpsimd.tensor_tensor(acc, acc, tmp[:], AL.add)


def build_nc():
    nc = bacc.Bacc(None, target_bir_lowering=False)
    x_ext = nc.declare_dram_parameter("x", [B_LOC, C, H, W], BF16, False)
    aq_ext = nc.declare_dram_parameter("aq", [C, INNER], BF16, False)
    ak_ext = nc.declare_dram_parameter("ak", [C, INNER], BF16, False)
    av_ext = nc.declare_dram_parameter("av", [C, INNER], BF16, False)
    w2_ext = nc.declare_dram_parameter("w2", [INNER, C], BF16, False)
    qt_ext = nc.declare_dram_parameter("qtap", [C, 9], F32, False)
    kt_ext = nc.declare_dram_parameter("kvtap", [C, 9], F32, False)
    bq_ext = nc.declare_dram_parameter("bq", [INNER, 1], F32, False)
    bk_ext = nc.declare_dram_parameter("bk", [INNER, 1], F32, False)
    b2_ext = nc.declare_dram_parameter("b2", [C, 1], F32, False)
    out_ext = nc.declare_dram_parameter("out", [B_LOC, C, H, W], F32, True)

    from contextlib import ExitStack
    with tile.TileContext(nc) as tc, ExitStack() as ctx:
        wpool = ctx.enter_context(tc.tile_pool(name="weights", bufs=1))
        xstage = ctx.enter_context(tc.tile_pool(name="xs", bufs=3))
        xpool = ctx.enter_context(tc.tile_pool(name="xp", bufs=4))
        scratch = ctx.enter_context(tc.tile_pool(name="scratch", bufs=3))
        y1pool = ctx.enter_context(tc.tile_pool(name="y1", bufs=7))
        y2pool = ctx.enter_context(tc.tile_pool(name="y2", bufs=7))
        qpool = ctx.enter_context(tc.tile_pool(name="q", bufs=4))
        kpool = ctx.enter_context(tc.tile_pool(name="k", bufs=4))
        vpool = ctx.enter_context(tc.tile_pool(name="v", bufs=3))
        epool = ctx.enter_context(tc.tile_pool(name="et", bufs=6))
        rpool = ctx.enter_context(tc.tile_pool(name="recip", bufs=2))
        opool = ctx.enter_context(tc.tile_pool(name="outT", bufs=4))
        fpool = ctx.enter_context(tc.tile_pool(name="fin", bufs=3))
        ps2 = ctx.enter_context(tc.tile_pool(name="ps2", bufs=2, space="PSUM"))
        psd2 = ctx.enter_context(tc.tile_pool(name="psd2", bufs=2, space="PSUM"))

        # ---- load weights (persistent) ----
        def wload(ext, kc_, shape, dtype, tag):
            t = wpool.tile(shape, dtype, tag=f"{tag}{kc_}", name=f"{tag}{kc_}")
            nc.sync.dma_start(t[:], ext[kc_ * 128:(kc_ + 1) * 128, :])
            return t

        aq_sb = [wload(aq_ext, i, [128, INNER], BF16, "aq") for i in range(KC)]
        ak_sb = [wload(ak_ext, i, [128, INNER], BF16, "ak") for i in range(KC)]
        av_sb = [wload(av_ext, i, [128, INNER], BF16, "av") for i in range(KC)]
        w2_sb = [wload(w2_ext, i, [128, C], BF16, "w2") for i in range(MC)]
        qt_sb = [wload(qt_ext, i, [128, 9], F32, "qt") for i in range(KC)]
        kt_sb = [wload(kt_ext, i, [128, 9], F32, "kt") for i in range(KC)]
        bq_sb = [wload(bq_ext, i, [128, 1], F32, "bq") for i in range(MC)]
        bk_sb = [wload(bk_ext, i, [128, 1], F32, "bk") for i in range(MC)]
        b2_sb = [wload(b2_ext, i, [128, 1], F32, "b2") for i in range(MC)]

        # ones-masks for denominator matmuls
        maskA = wpool.tile([128, 128], BF16, tag="maskA", name="maskA")
        maskB = wpool.tile([128, 128], BF16, tag="maskB", name="maskB")
        nc.gpsimd.memset(maskA[:], 0.0)
        nc.gpsimd.memset(maskA[:, 0:64], 1.0)
        nc.gpsimd.memset(maskB[:], 0.0)
        nc.gpsimd.memset(maskB[:, 64:128], 1.0)

        def conv_pair(b01):
            """DMA x for batches (2*b01, 2*b01+1) + both depthwise convs.
            Returns ([y1p x KC], [y2p x KC]) 2-batch-packed tiles.
            x lands contiguously (fast DMA, scalar-engine queue) and is
            scattered into the padded layout by a DVE copy."""
            b = 2 * b01
            y1, y2 = [], []
            for kc_ in range(KC):
                xs = xstage.tile([128, 2 * HW], BF16, tag="xs", name="xs")
                src = x_ext[b:b + 2, kc_ * 128:(kc_ + 1) * 128, :, :]
                nc.scalar.dma_start(
                    xs[:].rearrange("p (b hw) -> p b hw", b=2),
                    src.rearrange("b c h w -> c b (h w)"))
                xp = xpool.tile([128, 2 * PADN], BF16, tag="xp", name="xp")
                xpv = xp[:].rearrange("p (b r c) -> p b r c", b=2, c=PADR)
                nc.gpsimd.memset(xpv[:, :, 0:1, :], 0.0)
                nc.gpsimd.memset(xpv[:, :, 33:34, :], 0.0)
                nc.gpsimd.memset(xpv[:, :, 1:33, 0:1], 0.0)
                nc.gpsimd.memset(xpv[:, :, 1:33, 33:34], 0.0)
                nc.vector.tensor_copy(
                    xpv[:, :, 1:33, 1:33],
                    xs[:].rearrange("p (b h w) -> p b h w", b=2, w=W))
                a1 = y1pool.tile([128, 2 * HW], BF16, tag="y1", name="y1")
                _conv_s1(nc, scratch, xpv, qt_sb[kc_], a1[:])
                a2 = y2pool.tile([128, 2 * JK], BF16, tag="y2", name="y2")
                _conv_s2(nc, scratch, xpv, kt_sb[kc_], a2[:])
                y1.append(a1)
                y2.append(a2)
            return y1, y2

        def rest_phase(b, y1p, y2p):
            # views of this batch's halves inside the packed conv outputs
            h = b % 2
            y1 = [t[:, h * HW:(h + 1) * HW] for t in y1p]
            y2 = [t[:, h * JK:(h + 1) * JK] for t in y2p]
            # ---- stage A: q = Aq^T.T @ y1 + bq ----
            q_sb = []
            for mc_ in range(MC):
                qt = qpool.tile([128, HW], BF16, tag="q", name="qsb")
                ps = ps2.tile([128, 1024], F32, tag="ps2", name="psA")
                for n2 in range(2):
                    for kc_ in range(KC):
                        nc.tensor.matmul(
                            ps[:, n2 * 512:(n2 + 1) * 512],
                            aq_sb[kc_][:, mc_ * 128:(mc_ + 1) * 128],
                            y1[kc_][:, n2 * 512:(n2 + 1) * 512],
                            start=(kc_ == 0), stop=(kc_ == KC - 1))
                nc.scalar.activation(qt[:], ps[:], AF.Identity,
                                     bias=bq_sb[mc_][:], scale=1.0)
                q_sb.append(qt)

            # ---- stage Bk ----
            k_sb = []
            for mc_ in range(MC):
                kt = kpool.tile([128, JK], BF16, tag="k", name="ksb")
                ps = psd2.tile([128, JK], F32, tag="psd2", name="psBk")
                for kc_ in range(KC):
                    nc.tensor.matmul(
                        ps[:], ak_sb[kc_][:, mc_ * 128:(mc_ + 1) * 128], y2[kc_],
                        start=(kc_ == 0), stop=(kc_ == KC - 1))
                nc.scalar.activation(kt[:], ps[:], AF.Identity,
                                     bias=bk_sb[mc_][:], scale=1.0)
                k_sb.append(kt)

            # ---- stage Bv: vT[j, hd] ----
            vT_sb = []
            for jc in range(2):
                vt = vpool.tile([128, INNER], BF16, tag="v", name="vsb")
                ps = psd2.tile([128, INNER], F32, tag="psd2", name="psBv")
                for kc_ in range(KC):
                    nc.tensor.matmul(
                        ps[:], y2[kc_][:, jc * 128:(jc + 1) * 128], av_sb[kc_][:],
                        start=(kc_ == 0), stop=(kc_ == KC - 1))
                nc.scalar.activation(vt[:], ps[:], AF.Copy)
                vT_sb.append(vt)

            # ---- attention per head pair ----
            outT_sb = []
            for p in range(NPAIR):
                et = [[epool.tile([128, HW], BF16, tag="et", name="et")
                       for _ in range(2)] for _ in range(2)]
                for h01 in range(2):
                    hs = h01 * 64
                    for jc in range(2):
                        psd = psd2.tile([128, 1024], F32, tag="psd2", name="psd")
                        for ic in range(2):
                            nc.tensor.matmul(
                                psd[:, ic * 512:(ic + 1) * 512],
                                k_sb[p][hs:hs + 64, jc * 128:(jc + 1) * 128],
                                q_sb[p][hs:hs + 64, ic * 512:(ic + 1) * 512],
                                start=True, stop=True,
                                tile_position=(hs, 0))
                        nc.scalar.activation(et[h01][jc][:], psd[:],
                                             AF.Exp, scale=SCALE)

                # denominators (replicated over partitions) + reciprocal
                rec = rpool.tile([128, HW], F32, tag="recip", name="recip")
                psn = ps2.tile([128, 1024], F32, tag="ps2", name="psn")
                mms = [(maskA, et[0][0]), (maskA, et[0][1]),
                       (maskB, et[1][0]), (maskB, et[1][1])]
                for ic in range(2):
                    for mi, (msk, e) in enumerate(mms):
                        nc.tensor.matmul(
                            psn[:, ic * 512:(ic + 1) * 512], msk[:],
                            e[:, ic * 512:(ic + 1) * 512],
                            start=(mi == 0), stop=(mi == len(mms) - 1))
                nc.vector.reciprocal_approx_fast(out=rec[:], in_=psn[:])

                # stage D: outT = vT.T @ expT (col-packed head pair) + normalize
                ot = opool.tile([128, HW], BF16, tag="outT", name="outT")
                pso = ps2.tile([128, 1024], F32, tag="ps2", name="pso")
                for ic in range(2):
                    for h01 in range(2):
                        hs = h01 * 64
                        for jc in range(2):
                            nc.tensor.matmul(
                                pso[hs:hs + 64, ic * 512:(ic + 1) * 512],
                                vT_sb[jc][:, p * 128 + hs:p * 128 + hs + 64],
                                et[h01][jc][:, ic * 512:(ic + 1) * 512],
                                start=(jc == 0), stop=(jc == 1),
                                tile_position=(0, hs))
                nc.vector.tensor_tensor(ot[:], pso[:], rec[:], AL.mult)
                outT_sb.append(ot)

            # ---- stage E ----
            for mc_ in range(MC):
                fin = fpool.tile([128, HW], F32, tag="fin", name="fin")
                ps = ps2.tile([128, 1024], F32, tag="ps2", name="psE")
                for n2 in range(2):
                    for p in range(NPAIR):
                        nc.tensor.matmul(
                            ps[:, n2 * 512:(n2 + 1) * 512],
                            w2_sb[p][:, mc_ * 128:(mc_ + 1) * 128],
                            outT_sb[p][:, n2 * 512:(n2 + 1) * 512],
                            start=(p == 0), stop=(p == NPAIR - 1))
                nc.scalar.activation(fin[:], ps[:], AF.Identity,
                                     bias=b2_sb[mc_][:], scale=1.0)
                nc.sync.dma_start(
                    out_ext[b, mc_ * 128:(mc_ + 1) * 128, :, :],
                    fin[:].rearrange("p (h w) -> p h w", w=W))

        # emission order: conv (2-batch pairs) runs ahead so DVE/gpsimd feed
        # PE continuously (keeps the HAM clock warm)
        ys0 = conv_pair(0)
        rest_phase(0, *ys0)
        ys1 = conv_pair(1)
        rest_phase(1, *ys0)
        rest_phase(2, *ys1)
        rest_phase(3, *ys1)

    nc.compile()
    return nc


_NC_CACHE = None


def _get_nc():
    global _NC_CACHE
    if _NC_CACHE is None:
        _NC_CACHE = build_nc()
    return _NC_CACHE


def _prep_host(inputs):
    """Fold BN into pointwise weights; fold v-bias into final bias."""
    f32 = np.float32
    bf16 = ml_dtypes.bfloat16
    inv_q = (inputs['q_gamma'] / np.sqrt(inputs['q_var'] + EPS)).astype(f32)
    sh_q = (inputs['q_beta'] - inputs['q_mean'] * inv_q).astype(f32)
    A_q = (inputs['q_pw'] * inv_q[None, :]).astype(f32)
    b_q = (inputs['q_pw'].astype(f32) @ sh_q).astype(f32)

    inv_kv = (inputs['kv_gamma'] / np.sqrt(inputs['kv_var'] + EPS)).astype(f32)
    sh_kv = (inputs['kv_beta'] - inputs['kv_mean'] * inv_kv).astype(f32)
    A_kv = (inputs['kv_pw'] * inv_kv[None, :]).astype(f32)
    b_kv = (inputs['kv_pw'].astype(f32) @ sh_kv).astype(f32)
    A_k, A_v = A_kv[:INNER], A_kv[INNER:]
    b_k, b_v = b_kv[:INNER], b_kv[INNER:]

    W2 = inputs['out_w'].astype(f32)
    b2 = (inputs['out_b'].astype(f32) + W2 @ b_v).astype(f32)

    return {
        'aq': np.ascontiguousarray(A_q.T).astype(bf16),
        'ak': np.ascontiguousarray(A_k.T).astype(bf16),
        'av': np.ascontiguousarray(A_v.T).astype(bf16),
        'w2': np.ascontiguousarray(W2.T).astype(bf16),
        'qtap': np.ascontiguousarray(inputs['q_dw'].reshape(C, 9)).astype(f32),
        'kvtap': np.ascontiguousarray(inputs['kv_dw'].reshape(C, 9)).astype(f32),
        'bq': b_q.reshape(INNER, 1),
        'bk': b_k.reshape(INNER, 1),
        'b2': b2.reshape(C, 1),
    }


def kernel(**inputs):
    inputs = {k: np.asarray(v) for k, v in inputs.items()}
    nc = _get_nc()
    wmap = _prep_host(inputs)
    xb = inputs['x'].astype(ml_dtypes.bfloat16)
    in_maps = []
    for c in range(N_CORES):
        m = dict(wmap)
        m['x'] = np.ascontiguousarray(xb[c * B_LOC:(c + 1) * B_LOC])
        in_maps.append(m)
    res = run_bass_kernel_spmd(nc, in_maps, core_ids=list(range(N_CORES)))
    shards = [res.results[i]['out'] for i in range(N_CORES)]
    return np.concatenate(shards, axis=0).astype(np.float32)


# revision 27
# speedup vs baseline: 1.0461x; 1.0461x over previous
"""Trainium2 Bass kernel for nn_Attention_10582799417937.

Data-parallel over batch (32 -> 4 per core x 8 cores), weights replicated.
Per-core pipeline (per batch):
  depthwise 3x3 convs (q-path on DVE/ACT, kv-path on gpsimd)
  -> pointwise projections (PE matmuls, bf16)
  -> attention computed transposed: dotsT[j,i] = k_h^T . q_h^T so the
     attention matrix never needs transposing; softmax denominators via
     ones-mask matmuls on PE (replicated across partitions), normalize on DVE.
  -> output projection (PE) -> DRAM.
All BN affine and the V-path bias are folded into weights on the host.
"""
import sys
import numpy as np
import ml_dtypes

sys.path.insert(0, "/opt/trn_rl_repo")

import concourse.bass as bass
import concourse.mybir as mybir
import concourse.tile as tile
from concourse import bacc
from concourse.bass_utils import run_bass_kernel_spmd

# ---- problem constants (hardcoded per spec) ----
B, C, H, W = 32, 384, 32, 32
HEADS, D = 6, 64
INNER = HEADS * D          # 384
SCALE = D ** -0.5
EPS = 1e-5
N_CORES = 8
B_LOC = B // N_CORES       # 4
HW = H * W                 # 1024
HK, WK = H // 2, W // 2
JK = HK * WK               # 256
KC = C // 128              # 3 channel chunks
MC = INNER // 128          # 3 inner chunks (also head pairs)
NPAIR = HEADS // 2         # 3

BF16 = mybir.dt.bfloat16
F32 = mybir.dt.float32
AL = mybir.AluOpType
AF = mybir.ActivationFunctionType

# padded layout: [128, 34 rows x 34 cols]; data at rows 1..32, cols 1..32.
# Two batches are packed per xp tile ([128, 2, 34, 34]).
PADR = 34
PADN = PADR * PADR

# q-conv tap split: PE taps run as diagonal-matrix matmuls accumulating in
# PSUM; DVE taps run as a multiply/add chain; one tap pair joins on gpsimd.
PE_TAPS = [(1, 1), (1, 0), (1, 2), (0, 1), (2, 1)]
DVE_TAPS = [(0, 0), (0, 2), (2, 0), (2, 2)]
GP_PAIR = ((2, 0), (2, 2))

# kv tap order: center tap first (initializes accumulator)
ORDER = [(1, 1), (1, 0), (1, 2), (0, 0), (0, 1), (0, 2), (2, 0), (2, 1), (2, 2)]


def _conv_s1(nc, tc, pools, xpv, h, kc_, y1t):
    """Stride-1 3x3 depthwise conv for one batch (h of the xp pair).
    PE_TAPS accumulate diag(w_t) @ x_window into PSUM; DVE_TAPS build a
    partial on DVE (+one gpsimd join); the final DVE tensor_tensor add
    evicts psum + joins the partial into y1t ([128, 1024] bf16)."""
    scratch, psc, qdiag_sb, qt_sb = (pools["scratch"], pools["psc"],
                                     pools["qdiag"], pools["qt"])
    pst = psc.tile([128, HW], F32, tag="psc", name="pst")
    for ti, (dy, dx) in enumerate(PE_TAPS):
        dg = qdiag_sb[ti * KC + kc_]
        for n2 in range(2):
            rhs = xpv[:, h, dy + n2 * 16:dy + n2 * 16 + 16, dx:dx + W]
            nc.tensor.matmul(
                pst[:, n2 * 512:(n2 + 1) * 512], dg[:], rhs,
                start=(ti == 0), stop=(ti == len(PE_TAPS) - 1))
    # DVE partial over the remaining taps
    p = scratch.tile([128, HW], BF16, tag="cscr", name="cp")
    gp_tmp = None
    for idx, (dy, dx) in enumerate(DVE_TAPS):
        t = dy * 3 + dx
        scal = qt_sb[kc_][:, t:t + 1]
        win = xpv[:, h, dy:dy + 32, dx:dx + W]
        if idx == 0:
            nc.vector.tensor_scalar_mul(p[:], win, scal)
            continue
        tmp = scratch.tile([128, HW], BF16, tag="cscr", name="ct")
        nc.vector.tensor_scalar_mul(tmp[:], win, scal)
        if (dy, dx) in GP_PAIR:
            if gp_tmp is None:
                gp_tmp = tmp
                continue
            nc.gpsimd.tensor_tensor(gp_tmp[:], gp_tmp[:], tmp[:], AL.add)
            tmp = gp_tmp
        nc.vector.tensor_tensor(p[:], p[:], tmp[:], AL.add)
    # evict psum + join partial
    nc.vector.tensor_tensor(y1t, pst[:], p[:], AL.add)


def _conv_s2(nc, scratch, xpv, taps_sb, acc):
    """Stride-2 3x3 depthwise conv, 2-batch packed. DVE multiplies (per
    batch) feed a serial gpsimd accumulate chain. acc: [128, 512] bf16."""
    for idx, (dy, dx) in enumerate(ORDER):
        t = dy * 3 + dx
        scal = taps_sb[:, t:t + 1]
        if idx == 0:
            for h in range(2):
                nc.vector.tensor_scalar_mul(
                    acc[:, h * JK:(h + 1) * JK],
                    xpv[:, h, dy:dy + 31:2, dx:dx + 31:2], scal)
            continue
        tmp = scratch.tile([128, 2 * JK], BF16, tag="kscr", name="kscr")
        for h in range(2):
            nc.vector.tensor_scalar_mul(
                tmp[:, h * JK:(h + 1) * JK],
                xpv[:, h, dy:dy + 31:2, dx:dx + 31:2], scal)
        nc.gpsimd.tensor_tensor(acc, acc, tmp[:], AL.add)


def build_nc():
    nc = bacc.Bacc(None, target_bir_lowering=False)
    x_ext = nc.declare_dram_parameter("x", [B_LOC, C, H, W], BF16, False)
    aq_ext = nc.declare_dram_parameter("aq", [C, INNER], BF16, False)
    ak_ext = nc.declare_dram_parameter("ak", [C, INNER], BF16, False)
    av_ext = nc.declare_dram_parameter("av", [C, INNER], BF16, False)
    w2_ext = nc.declare_dram_parameter("w2", [INNER, C], BF16, False)
    qt_ext = nc.declare_dram_parameter("qtap", [C, 9], F32, False)
    kt_ext = nc.declare_dram_parameter("kvtap", [C, 9], F32, False)
    qd_ext = nc.declare_dram_parameter("qdiag", [len(PE_TAPS) * C, 128], BF16,
                                       False)
    bq_ext = nc.declare_dram_parameter("bq", [INNER, 1], F32, False)
    bk_ext = nc.declare_dram_parameter("bk", [INNER, 1], F32, False)
    b2_ext = nc.declare_dram_parameter("b2", [C, 1], F32, False)
    out_ext = nc.declare_dram_parameter("out", [B_LOC, C, H, W], F32, True)

    from contextlib import ExitStack
    with tile.TileContext(nc) as tc, ExitStack() as ctx:
        wpool = ctx.enter_context(tc.tile_pool(name="weights", bufs=1))
        xstage = ctx.enter_context(tc.tile_pool(name="xs", bufs=3))
        xpool = ctx.enter_context(tc.tile_pool(name="xp", bufs=4))
        scratch = ctx.enter_context(tc.tile_pool(name="scratch", bufs=3))
        y1pool = ctx.enter_context(tc.tile_pool(name="y1", bufs=14))
        y2pool = ctx.enter_context(tc.tile_pool(name="y2", bufs=7))
        qpool = ctx.enter_context(tc.tile_pool(name="q", bufs=4))
        kpool = ctx.enter_context(tc.tile_pool(name="k", bufs=4))
        vpool = ctx.enter_context(tc.tile_pool(name="v", bufs=3))
        epool = ctx.enter_context(tc.tile_pool(name="et", bufs=6))
        rpool = ctx.enter_context(tc.tile_pool(name="recip", bufs=2))
        opool = ctx.enter_context(tc.tile_pool(name="outT", bufs=4))
        fpool = ctx.enter_context(tc.tile_pool(name="fin", bufs=3))
        ps2 = ctx.enter_context(tc.tile_pool(name="ps2", bufs=2, space="PSUM"))
        psc = ctx.enter_context(tc.tile_pool(name="psc", bufs=2, space="PSUM"))

        # ---- load weights (persistent) ----
        def wload(ext, kc_, shape, dtype, tag):
            t = wpool.tile(shape, dtype, tag=f"{tag}{kc_}", name=f"{tag}{kc_}")
            nc.sync.dma_start(t[:], ext[kc_ * 128:(kc_ + 1) * 128, :])
            return t

        aq_sb = [wload(aq_ext, i, [128, INNER], BF16, "aq") for i in range(KC)]
        ak_sb = [wload(ak_ext, i, [128, INNER], BF16, "ak") for i in range(KC)]
        av_sb = [wload(av_ext, i, [128, INNER], BF16, "av") for i in range(KC)]
        w2_sb = [wload(w2_ext, i, [128, C], BF16, "w2") for i in range(MC)]
        qt_sb = [wload(qt_ext, i, [128, 9], F32, "qt") for i in range(KC)]
        kt_sb = [wload(kt_ext, i, [128, 9], F32, "kt") for i in range(KC)]
        bq_sb = [wload(bq_ext, i, [128, 1], F32, "bq") for i in range(MC)]
        bk_sb = [wload(bk_ext, i, [128, 1], F32, "bk") for i in range(MC)]
        b2_sb = [wload(b2_ext, i, [128, 1], F32, "b2") for i in range(MC)]
        qdiag_sb = [wload(qd_ext, i, [128, 128], BF16, "qd")
                    for i in range(len(PE_TAPS) * KC)]

        # ones-masks for denominator matmuls
        maskA = wpool.tile([128, 128], BF16, tag="maskA", name="maskA")
        maskB = wpool.tile([128, 128], BF16, tag="maskB", name="maskB")
        nc.gpsimd.memset(maskA[:], 0.0)
        nc.gpsimd.memset(maskA[:, 0:64], 1.0)
        nc.gpsimd.memset(maskB[:], 0.0)
        nc.gpsimd.memset(maskB[:, 64:128], 1.0)

        def conv_pair(b01):
            """DMA x for batches (2*b01, 2*b01+1) + both depthwise convs.
            Returns ([y1p x KC], [y2p x KC]) 2-batch-packed tiles.
            x lands contiguously (fast DMA, scalar-engine queue) and is
            scattered into the padded layout by a DVE copy."""
            b = 2 * b01
            y1, y2 = [], []
            for kc_ in range(KC):
                xs = xstage.tile([128, 2 * HW], BF16, tag="xs", name="xs")
                src = x_ext[b:b + 2, kc_ * 128:(kc_ + 1) * 128, :, :]
                nc.scalar.dma_start(
                    xs[:].rearrange("p (b hw) -> p b hw", b=2),
                    src.rearrange("b c h w -> c b (h w)"))
                xp = xpool.tile([128, 2 * PADN], BF16, tag="xp", name="xp")
                xpv = xp[:].rearrange("p (b r c) -> p b r c", b=2, c=PADR)
                nc.gpsimd.memset(xpv[:, :, 0:1, :], 0.0)
                nc.gpsimd.memset(xpv[:, :, 33:34, :], 0.0)
                nc.gpsimd.memset(xpv[:, :, 1:33, 0:1], 0.0)
                nc.gpsimd.memset(xpv[:, :, 1:33, 33:34], 0.0)
                nc.vector.tensor_copy(
                    xpv[:, :, 1:33, 1:33],
                    xs[:].rearrange("p (b h w) -> p b h w", b=2, w=W))
                pools = {"scratch": scratch, "psc": psc, "qdiag": qdiag_sb,
                         "qt": qt_sb}
                pair = []
                for h in range(2):
                    a1 = y1pool.tile([128, HW], BF16, tag="y1", name="y1")
                    _conv_s1(nc, tc, pools, xpv, h, kc_, a1[:])
                    pair.append(a1)
                y1.append(pair)
                a2 = y2pool.tile([128, 2 * JK], BF16, tag="y2", name="y2")
                _conv_s2(nc, scratch, xpv, kt_sb[kc_], a2[:])
                y2.append(a2)
            return y1, y2

        def rest_phase(b, y1p, y2p):
            # views of this batch's half inside the packed conv outputs
            h = b % 2
            y1 = [pair[h][:] for pair in y1p]
            y2 = [t[:, h * JK:(h + 1) * JK] for t in y2p]
            # ---- stage A: q = Aq^T.T @ y1 + bq ----
            q_sb = []
            for mc_ in range(MC):
                qt = qpool.tile([128, HW], BF16, tag="q", name="qsb")
                ps = ps2.tile([128, 1024], F32, tag="ps2", name="psA")
                for n2 in range(2):
                    for kc_ in range(KC):
                        nc.tensor.matmul(
                            ps[:, n2 * 512:(n2 + 1) * 512],
                            aq_sb[kc_][:, mc_ * 128:(mc_ + 1) * 128],
                            y1[kc_][:, n2 * 512:(n2 + 1) * 512],
                            start=(kc_ == 0), stop=(kc_ == KC - 1))
                nc.scalar.activation(qt[:], ps[:], AF.Identity,
                                     bias=bq_sb[mc_][:], scale=1.0)
                q_sb.append(qt)

            # ---- stage Bk ----
            k_sb = []
            for mc_ in range(MC):
                kt = kpool.tile([128, JK], BF16, tag="k", name="ksb")
                ps = psc.tile([128, JK], F32, tag="psc", name="psBk")
                for kc_ in range(KC):
                    nc.tensor.matmul(
                        ps[:], ak_sb[kc_][:, mc_ * 128:(mc_ + 1) * 128], y2[kc_],
                        start=(kc_ == 0), stop=(kc_ == KC - 1))
                nc.scalar.activation(kt[:], ps[:], AF.Identity,
                                     bias=bk_sb[mc_][:], scale=1.0)
                k_sb.append(kt)

            # ---- stage Bv: vT[j, hd] ----
            vT_sb = []
            for jc in range(2):
                vt = vpool.tile([128, INNER], BF16, tag="v", name="vsb")
                ps = psc.tile([128, INNER], F32, tag="psc", name="psBv")
                for kc_ in range(KC):
                    nc.tensor.matmul(
                        ps[:], y2[kc_][:, jc * 128:(jc + 1) * 128], av_sb[kc_][:],
                        start=(kc_ == 0), stop=(kc_ == KC - 1))
                nc.scalar.activation(vt[:], ps[:], AF.Copy)
                vT_sb.append(vt)

            # ---- attention per head pair ----
            outT_sb = []
            for p in range(NPAIR):
                et = [[epool.tile([128, HW], BF16, tag="et", name="et")
                       for _ in range(2)] for _ in range(2)]
                for h01 in range(2):
                    hs = h01 * 64
                    for jc in range(2):
                        psd = ps2.tile([128, 1024], F32, tag="ps2", name="psd")
                        for ic in range(2):
                            nc.tensor.matmul(
                                psd[:, ic * 512:(ic + 1) * 512],
                                k_sb[p][hs:hs + 64, jc * 128:(jc + 1) * 128],
                                q_sb[p][hs:hs + 64, ic * 512:(ic + 1) * 512],
                                start=True, stop=True,
                                tile_position=(hs, 0))
                        nc.scalar.activation(et[h01][jc][:], psd[:],
                                             AF.Exp, scale=SCALE)

                # denominators (replicated over partitions) + reciprocal
                rec = rpool.tile([128, HW], F32, tag="recip", name="recip")
                psn = ps2.tile([128, 1024], F32, tag="ps2", name="psn")
                mms = [(maskA, et[0][0]), (maskA, et[0][1]),
                       (maskB, et[1][0]), (maskB, et[1][1])]
                for ic in range(2):
                    for mi, (msk, e) in enumerate(mms):
                        nc.tensor.matmul(
                            psn[:, ic * 512:(ic + 1) * 512], msk[:],
                            e[:, ic * 512:(ic + 1) * 512],
                            start=(mi == 0), stop=(mi == len(mms) - 1))
                nc.vector.reciprocal_approx_fast(out=rec[:], in_=psn[:])

                # stage D: outT = vT.T @ expT (col-packed head pair) + normalize
                ot = opool.tile([128, HW], BF16, tag="outT", name="outT")
                pso = ps2.tile([128, 1024], F32, tag="ps2", name="pso")
                for ic in range(2):
                    for h01 in range(2):
                        hs = h01 * 64
                        for jc in range(2):
                            nc.tensor.matmul(
                                pso[hs:hs + 64, ic * 512:(ic + 1) * 512],
                                vT_sb[jc][:, p * 128 + hs:p * 128 + hs + 64],
                                et[h01][jc][:, ic * 512:(ic + 1) * 512],
                                start=(jc == 0), stop=(jc == 1),
                                tile_position=(0, hs))
                nc.vector.tensor_tensor(ot[:], pso[:], rec[:], AL.mult)
                outT_sb.append(ot)

            # ---- stage E ----
            for mc_ in range(MC):
                fin = fpool.tile([128, HW], F32, tag="fin", name="fin")
                ps = ps2.tile([128, 1024], F32, tag="ps2", name="psE")
                for n2 in range(2):
                    for p in range(NPAIR):
                        nc.tensor.matmul(
                            ps[:, n2 * 512:(n2 + 1) * 512],
                            w2_sb[p][:, mc_ * 128:(mc_ + 1) * 128],
                            outT_sb[p][:, n2 * 512:(n2 + 1) * 512],
                            start=(p == 0), stop=(p == NPAIR - 1))
                nc.scalar.activation(fin[:], ps[:], AF.Identity,
                                     bias=b2_sb[mc_][:], scale=1.0)
                nc.sync.dma_start(
                    out_ext[b, mc_ * 128:(mc_ + 1) * 128, :, :],
                    fin[:].rearrange("p (h w) -> p h w", w=W))

        # emission order: conv (2-batch pairs) runs ahead so DVE/gpsimd feed
        # PE continuously (keeps the HAM clock warm)
        ys0 = conv_pair(0)
        rest_phase(0, *ys0)
        ys1 = conv_pair(1)
        rest_phase(1, *ys0)
        rest_phase(2, *ys1)
        rest_phase(3, *ys1)

    nc.compile()
    return nc


_NC_CACHE = None


def _get_nc():
    global _NC_CACHE
    if _NC_CACHE is None:
        _NC_CACHE = build_nc()
    return _NC_CACHE


def _prep_host(inputs):
    """Fold BN into pointwise weights; fold v-bias into final bias."""
    f32 = np.float32
    bf16 = ml_dtypes.bfloat16
    inv_q = (inputs['q_gamma'] / np.sqrt(inputs['q_var'] + EPS)).astype(f32)
    sh_q = (inputs['q_beta'] - inputs['q_mean'] * inv_q).astype(f32)
    A_q = (inputs['q_pw'] * inv_q[None, :]).astype(f32)
    b_q = (inputs['q_pw'].astype(f32) @ sh_q).astype(f32)

    inv_kv = (inputs['kv_gamma'] / np.sqrt(inputs['kv_var'] + EPS)).astype(f32)
    sh_kv = (inputs['kv_beta'] - inputs['kv_mean'] * inv_kv).astype(f32)
    A_kv = (inputs['kv_pw'] * inv_kv[None, :]).astype(f32)
    b_kv = (inputs['kv_pw'].astype(f32) @ sh_kv).astype(f32)
    A_k, A_v = A_kv[:INNER], A_kv[INNER:]
    b_k, b_v = b_kv[:INNER], b_kv[INNER:]

    W2 = inputs['out_w'].astype(f32)
    b2 = (inputs['out_b'].astype(f32) + W2 @ b_v).astype(f32)

    qtap = inputs['q_dw'].reshape(C, 9).astype(f32)
    qdiag = np.zeros((len(PE_TAPS) * C, 128), f32)
    for ti, (dy, dx) in enumerate(PE_TAPS):
        t = dy * 3 + dx
        for kc_ in range(KC):
            blk = np.diag(qtap[kc_ * 128:(kc_ + 1) * 128, t])
            qdiag[ti * C + kc_ * 128:ti * C + (kc_ + 1) * 128, :] = blk

    return {
        'qdiag': qdiag.astype(bf16),
        'aq': np.ascontiguousarray(A_q.T).astype(bf16),
        'ak': np.ascontiguousarray(A_k.T).astype(bf16),
        'av': np.ascontiguousarray(A_v.T).astype(bf16),
        'w2': np.ascontiguousarray(W2.T).astype(bf16),
        'qtap': np.ascontiguousarray(inputs['q_dw'].reshape(C, 9)).astype(f32),
        'kvtap': np.ascontiguousarray(inputs['kv_dw'].reshape(C, 9)).astype(f32),
        'bq': b_q.reshape(INNER, 1),
        'bk': b_k.reshape(INNER, 1),
        'b2': b2.reshape(C, 1),
    }


def kernel(**inputs):
    inputs = {k: np.asarray(v) for k, v in inputs.items()}
    nc = _get_nc()
    wmap = _prep_host(inputs)
    xb = inputs['x'].astype(ml_dtypes.bfloat16)
    in_maps = []
    for c in range(N_CORES):
        m = dict(wmap)
        m['x'] = np.ascontiguousarray(xb[c * B_LOC:(c + 1) * B_LOC])
        in_maps.append(m)
    res = run_bass_kernel_spmd(nc, in_maps, core_ids=list(range(N_CORES)))
    shards = [res.results[i]['out'] for i in range(N_CORES)]
    return np.concatenate(shards, axis=0).astype(np.float32)


# revision 31
# speedup vs baseline: 1.0518x; 1.0054x over previous
"""Trainium2 Bass kernel for nn_Attention_10582799417937.

Data-parallel over batch (32 -> 4 per core x 8 cores), weights replicated.
Per-core pipeline (per batch):
  depthwise 3x3 convs (q-path on DVE/ACT, kv-path on gpsimd)
  -> pointwise projections (PE matmuls, bf16)
  -> attention computed transposed: dotsT[j,i] = k_h^T . q_h^T so the
     attention matrix never needs transposing; softmax denominators via
     ones-mask matmuls on PE (replicated across partitions), normalize on DVE.
  -> output projection (PE) -> DRAM.
All BN affine and the V-path bias are folded into weights on the host.
"""
import sys
import numpy as np
import ml_dtypes

sys.path.insert(0, "/opt/trn_rl_repo")

import concourse.bass as bass
import concourse.mybir as mybir
import concourse.tile as tile
from concourse import bacc
from concourse.bass_utils import run_bass_kernel_spmd

# ---- problem constants (hardcoded per spec) ----
B, C, H, W = 32, 384, 32, 32
HEADS, D = 6, 64
INNER = HEADS * D          # 384
SCALE = D ** -0.5
EPS = 1e-5
N_CORES = 8
B_LOC = B // N_CORES       # 4
HW = H * W                 # 1024
HK, WK = H // 2, W // 2
JK = HK * WK               # 256
KC = C // 128              # 3 channel chunks
MC = INNER // 128          # 3 inner chunks (also head pairs)
NPAIR = HEADS // 2         # 3

BF16 = mybir.dt.bfloat16
F32 = mybir.dt.float32
AL = mybir.AluOpType
AF = mybir.ActivationFunctionType

# padded layout: [128, 34 rows x 34 cols]; data at rows 1..32, cols 1..32.
# Two batches are packed per xp tile ([128, 2, 34, 34]).
PADR = 34
PADN = PADR * PADR

# tap order: center tap first (initializes accumulator)
ORDER = [(1, 1), (1, 0), (1, 2), (0, 0), (0, 1), (0, 2), (2, 0), (2, 1), (2, 2)]


def _conv_s1_dve(nc, scratch, xpv, taps_sb, acc):
    """Stride-1 3x3 depthwise conv over a 2-batch packed tile on DVE.
    Per-batch multiplies (2-level APs keep the fast mode), 2-batch adds.
    acc: [128, 2048] bf16."""
    for idx, (dy, dx) in enumerate(ORDER):
        t = dy * 3 + dx
        scal = taps_sb[:, t:t + 1]
        if idx == 0:
            for h in range(2):
                nc.vector.tensor_scalar_mul(
                    acc[:, h * HW:(h + 1) * HW],
                    xpv[:, h, dy:dy + 32, dx:dx + W], scal)
            continue
        tmp = scratch.tile([128, 2 * HW], BF16, tag="cscr", name="cscr")
        for h in range(2):
            nc.vector.tensor_scalar_mul(
                tmp[:, h * HW:(h + 1) * HW],
                xpv[:, h, dy:dy + 32, dx:dx + W], scal)
        nc.vector.tensor_tensor(acc, acc, tmp[:], AL.add)


def _conv_s2_dve(nc, scratch, xpv, taps_sb, acc):
    """Stride-2 3x3 depthwise conv, 2-batch packed. DVE multiplies feed a
    serial gpsimd accumulate chain. acc: [128, 512] bf16."""
    for idx, (dy, dx) in enumerate(ORDER):
        t = dy * 3 + dx
        scal = taps_sb[:, t:t + 1]
        if idx == 0:
            for h in range(2):
                nc.vector.tensor_scalar_mul(
                    acc[:, h * JK:(h + 1) * JK],
                    xpv[:, h, dy:dy + 31:2, dx:dx + 31:2], scal)
            continue
        tmp = scratch.tile([128, 2 * JK], BF16, tag="kscr", name="kscr")
        for h in range(2):
            nc.vector.tensor_scalar_mul(
                tmp[:, h * JK:(h + 1) * JK],
                xpv[:, h, dy:dy + 31:2, dx:dx + 31:2], scal)
        nc.gpsimd.tensor_tensor(acc, acc, tmp[:], AL.add)


def _conv_s1_pe(nc, psc, qdiag_sb, xpv, h, kc_, y1t):
    """Stride-1 conv for one batch entirely on PE: 9 accumulating
    diag(w_t) @ x_window matmuls, ACT evict -> y1t [128, 1024] bf16."""
    pst = psc.tile([128, HW], F32, tag="psc", name="pst")
    for ti, (dy, dx) in enumerate(ORDER):
        dg = qdiag_sb[ti * KC + kc_]
        for n2 in range(2):
            rhs = xpv[:, h, dy + n2 * 16:dy + n2 * 16 + 16, dx:dx + W]
            nc.tensor.matmul(
                pst[:, n2 * 512:(n2 + 1) * 512], dg[:], rhs,
                start=(ti == 0), stop=(ti == len(ORDER) - 1))
    nc.scalar.activation(y1t, pst[:], AF.Copy)


def _conv_s2_pe(nc, psc, kvdiag_sb, xpv, h, kc_, y2t):
    """Stride-2 conv for one batch on PE: 9 accumulating matmuls with
    stride-2 windows, ACT evict -> y2t [128, 256] bf16."""
    pst = psc.tile([128, JK], F32, tag="psc", name="pstk")
    for ti, (dy, dx) in enumerate(ORDER):
        dg = kvdiag_sb[ti * KC + kc_]
        rhs = xpv[:, h, dy:dy + 31:2, dx:dx + 31:2]
        nc.tensor.matmul(pst[:], dg[:], rhs,
                         start=(ti == 0), stop=(ti == len(ORDER) - 1))
    nc.scalar.activation(y2t, pst[:], AF.Copy)


def build_nc():
    nc = bacc.Bacc(None, target_bir_lowering=False)
    x_ext = nc.declare_dram_parameter("x", [B_LOC, C, H, W], BF16, False)
    aq_ext = nc.declare_dram_parameter("aq", [C, INNER], BF16, False)
    ak_ext = nc.declare_dram_parameter("ak", [C, INNER], BF16, False)
    av_ext = nc.declare_dram_parameter("av", [C, INNER], BF16, False)
    w2_ext = nc.declare_dram_parameter("w2", [INNER, C], BF16, False)
    qt_ext = nc.declare_dram_parameter("qtap", [C, 9], F32, False)
    kt_ext = nc.declare_dram_parameter("kvtap", [C, 9], F32, False)
    qd_ext = nc.declare_dram_parameter("qdiag", [9 * C, 128], BF16, False)
    kd_ext = nc.declare_dram_parameter("kvdiag", [9 * C, 128], BF16, False)
    bq_ext = nc.declare_dram_parameter("bq", [INNER, 1], F32, False)
    bk_ext = nc.declare_dram_parameter("bk", [INNER, 1], F32, False)
    b2_ext = nc.declare_dram_parameter("b2", [C, 1], F32, False)
    out_ext = nc.declare_dram_parameter("out", [B_LOC, C, H, W], F32, True)

    from contextlib import ExitStack
    with tile.TileContext(nc) as tc, ExitStack() as ctx:
        wpool = ctx.enter_context(tc.tile_pool(name="weights", bufs=1))
        xstage = ctx.enter_context(tc.tile_pool(name="xs", bufs=2))
        xpool = ctx.enter_context(tc.tile_pool(name="xp", bufs=3))
        scratch = ctx.enter_context(tc.tile_pool(name="scratch", bufs=2))
        y1pool = ctx.enter_context(tc.tile_pool(name="y1", bufs=3))
        y2pool = ctx.enter_context(tc.tile_pool(name="y2", bufs=3))
        qpool = ctx.enter_context(tc.tile_pool(name="q", bufs=4))
        kpool = ctx.enter_context(tc.tile_pool(name="k", bufs=4))
        vpool = ctx.enter_context(tc.tile_pool(name="v", bufs=3))
        epool = ctx.enter_context(tc.tile_pool(name="et", bufs=6))
        rpool = ctx.enter_context(tc.tile_pool(name="recip", bufs=2))
        opool = ctx.enter_context(tc.tile_pool(name="outT", bufs=4))
        fpool = ctx.enter_context(tc.tile_pool(name="fin", bufs=2))
        ps2 = ctx.enter_context(tc.tile_pool(name="ps2", bufs=2, space="PSUM"))
        psc = ctx.enter_context(tc.tile_pool(name="psc", bufs=2, space="PSUM"))

        # ---- load weights (persistent) ----
        def wload(ext, kc_, shape, dtype, tag):
            t = wpool.tile(shape, dtype, tag=f"{tag}{kc_}", name=f"{tag}{kc_}")
            nc.sync.dma_start(t[:], ext[kc_ * 128:(kc_ + 1) * 128, :])
            return t

        aq_sb = [wload(aq_ext, i, [128, INNER], BF16, "aq") for i in range(KC)]
        ak_sb = [wload(ak_ext, i, [128, INNER], BF16, "ak") for i in range(KC)]
        av_sb = [wload(av_ext, i, [128, INNER], BF16, "av") for i in range(KC)]
        w2_sb = [wload(w2_ext, i, [128, C], BF16, "w2") for i in range(MC)]
        qt_sb = [wload(qt_ext, i, [128, 9], F32, "qt") for i in range(KC)]
        kt_sb = [wload(kt_ext, i, [128, 9], F32, "kt") for i in range(KC)]
        bq_sb = [wload(bq_ext, i, [128, 1], F32, "bq") for i in range(MC)]
        bk_sb = [wload(bk_ext, i, [128, 1], F32, "bk") for i in range(MC)]
        b2_sb = [wload(b2_ext, i, [128, 1], F32, "b2") for i in range(MC)]
        qdiag_sb = [wload(qd_ext, i, [128, 128], BF16, "qd")
                    for i in range(9 * KC)]
        kvdiag_sb = [wload(kd_ext, i, [128, 128], BF16, "kd")
                     for i in range(9 * KC)]

        # ones-masks for denominator matmuls
        maskA = wpool.tile([128, 128], BF16, tag="maskA", name="maskA")
        maskB = wpool.tile([128, 128], BF16, tag="maskB", name="maskB")
        nc.gpsimd.memset(maskA[:], 0.0)
        nc.gpsimd.memset(maskA[:, 0:64], 1.0)
        nc.gpsimd.memset(maskB[:], 0.0)
        nc.gpsimd.memset(maskB[:, 64:128], 1.0)

        def conv_pair(b01, on_pe):
            """DMA x for batches (2*b01, 2*b01+1) + both depthwise convs.
            on_pe selects the PE (diag matmul) or DVE implementation.
            Returns (y1 per-batch lists, y2 per-batch lists) of APs."""
            b = 2 * b01
            y1 = [[], []]
            y2 = [[], []]
            for kc_ in range(KC):
                xs = xstage.tile([128, 2 * HW], BF16, tag="xs", name="xs")
                src = x_ext[b:b + 2, kc_ * 128:(kc_ + 1) * 128, :, :]
                nc.scalar.dma_start(
                    xs[:].rearrange("p (b hw) -> p b hw", b=2),
                    src.rearrange("b c h w -> c b (h w)"))
                xp = xpool.tile([128, 2 * PADN], BF16, tag="xp", name="xp")
                xpv = xp[:].rearrange("p (b r c) -> p b r c", b=2, c=PADR)
                nc.gpsimd.memset(xpv[:, :, 0:1, :], 0.0)
                nc.gpsimd.memset(xpv[:, :, 33:34, :], 0.0)
                nc.gpsimd.memset(xpv[:, :, 1:33, 0:1], 0.0)
                nc.gpsimd.memset(xpv[:, :, 1:33, 33:34], 0.0)
                nc.vector.tensor_copy(
                    xpv[:, :, 1:33, 1:33],
                    xs[:].rearrange("p (b h w) -> p b h w", b=2, w=W))
                if on_pe:
                    for h in range(2):
                        a1 = y1pool.tile([128, HW], BF16, tag="y1", name="y1", bufs=6)
                        _conv_s1_pe(nc, psc, qdiag_sb, xpv, h, kc_, a1[:])
                        y1[h].append(a1[:])
                        a2 = y2pool.tile([128, JK], BF16, tag="y2", name="y2", bufs=6)
                        _conv_s2_pe(nc, psc, kvdiag_sb, xpv, h, kc_, a2[:])
                        y2[h].append(a2[:])
                else:
                    a1 = y1pool.tile([128, 2 * HW], BF16, tag="y1p", name="y1p")
                    _conv_s1_dve(nc, scratch, xpv, qt_sb[kc_], a1[:])
                    a2 = y2pool.tile([128, 2 * JK], BF16, tag="y2p", name="y2p")
                    _conv_s2_dve(nc, scratch, xpv, kt_sb[kc_], a2[:])
                    for h in range(2):
                        y1[h].append(a1[:, h * HW:(h + 1) * HW])
                        y2[h].append(a2[:, h * JK:(h + 1) * JK])
            return y1, y2

        def rest_phase(b, y1, y2):
            # ---- stage A: q = Aq^T.T @ y1 + bq ----
            q_sb = []
            for mc_ in range(MC):
                qt = qpool.tile([128, HW], BF16, tag="q", name="qsb")
                ps = ps2.tile([128, 1024], F32, tag="ps2", name="psA")
                for n2 in range(2):
                    for kc_ in range(KC):
                        nc.tensor.matmul(
                            ps[:, n2 * 512:(n2 + 1) * 512],
                            aq_sb[kc_][:, mc_ * 128:(mc_ + 1) * 128],
                            y1[kc_][:, n2 * 512:(n2 + 1) * 512],
                            start=(kc_ == 0), stop=(kc_ == KC - 1))
                nc.scalar.activation(qt[:], ps[:], AF.Identity,
                                     bias=bq_sb[mc_][:], scale=1.0)
                q_sb.append(qt)

            # ---- stage Bk ----
            k_sb = []
            for mc_ in range(MC):
                kt = kpool.tile([128, JK], BF16, tag="k", name="ksb")
                ps = psc.tile([128, JK], F32, tag="psc", name="psBk")
                for kc_ in range(KC):
                    nc.tensor.matmul(
                        ps[:], ak_sb[kc_][:, mc_ * 128:(mc_ + 1) * 128], y2[kc_],
                        start=(kc_ == 0), stop=(kc_ == KC - 1))
                nc.scalar.activation(kt[:], ps[:], AF.Identity,
                                     bias=bk_sb[mc_][:], scale=1.0)
                k_sb.append(kt)

            # ---- stage Bv: vT[j, hd] ----
            vT_sb = []
            for jc in range(2):
                vt = vpool.tile([128, INNER], BF16, tag="v", name="vsb")
                ps = psc.tile([128, INNER], F32, tag="psc", name="psBv")
                for kc_ in range(KC):
                    nc.tensor.matmul(
                        ps[:], y2[kc_][:, jc * 128:(jc + 1) * 128], av_sb[kc_][:],
                        start=(kc_ == 0), stop=(kc_ == KC - 1))
                nc.scalar.activation(vt[:], ps[:], AF.Copy)
                vT_sb.append(vt)

            # ---- attention per head pair ----
            outT_sb = []
            for p in range(NPAIR):
                et = [[epool.tile([128, HW], BF16, tag="et", name="et")
                       for _ in range(2)] for _ in range(2)]
                for h01 in range(2):
                    hs = h01 * 64
                    for jc in range(2):
                        psd = ps2.tile([128, 1024], F32, tag="ps2", name="psd")
                        for ic in range(2):
                            nc.tensor.matmul(
                                psd[:, ic * 512:(ic + 1) * 512],
                                k_sb[p][hs:hs + 64, jc * 128:(jc + 1) * 128],
                                q_sb[p][hs:hs + 64, ic * 512:(ic + 1) * 512],
                                start=True, stop=True,
                                tile_position=(hs, 0))
                        nc.scalar.activation(et[h01][jc][:], psd[:],
                                             AF.Exp, scale=SCALE)

                # denominators (replicated over partitions) + reciprocal
                rec = rpool.tile([128, HW], F32, tag="recip", name="recip")
                psn = ps2.tile([128, 1024], F32, tag="ps2", name="psn")
                mms = [(maskA, et[0][0]), (maskA, et[0][1]),
                       (maskB, et[1][0]), (maskB, et[1][1])]
                for ic in range(2):
                    for mi, (msk, e) in enumerate(mms):
                        nc.tensor.matmul(
                            psn[:, ic * 512:(ic + 1) * 512], msk[:],
                            e[:, ic * 512:(ic + 1) * 512],
                            start=(mi == 0), stop=(mi == len(mms) - 1))
                nc.vector.reciprocal_approx_fast(out=rec[:], in_=psn[:])

                # stage D: outT = vT.T @ expT (col-packed head pair) + normalize
                ot = opool.tile([128, HW], BF16, tag="outT", name="outT")
                pso = ps2.tile([128, 1024], F32, tag="ps2", name="pso")
                for ic in range(2):
                    for h01 in range(2):
                        hs = h01 * 64
                        for jc in range(2):
                            nc.tensor.matmul(
                                pso[hs:hs + 64, ic * 512:(ic + 1) * 512],
                                vT_sb[jc][:, p * 128 + hs:p * 128 + hs + 64],
                                et[h01][jc][:, ic * 512:(ic + 1) * 512],
                                start=(jc == 0), stop=(jc == 1),
                                tile_position=(0, hs))
                nc.vector.tensor_tensor(ot[:], pso[:], rec[:], AL.mult)
                outT_sb.append(ot)

            # ---- stage E ----
            for mc_ in range(MC):
                fin = fpool.tile([128, HW], F32, tag="fin", name="fin")
                ps = ps2.tile([128, 1024], F32, tag="ps2", name="psE")
                for n2 in range(2):
                    for p in range(NPAIR):
                        nc.tensor.matmul(
                            ps[:, n2 * 512:(n2 + 1) * 512],
                            w2_sb[p][:, mc_ * 128:(mc_ + 1) * 128],
                            outT_sb[p][:, n2 * 512:(n2 + 1) * 512],
                            start=(p == 0), stop=(p == NPAIR - 1))
                nc.scalar.activation(fin[:], ps[:], AF.Identity,
                                     bias=b2_sb[mc_][:], scale=1.0)
                nc.sync.dma_start(
                    out_ext[b, mc_ * 128:(mc_ + 1) * 128, :, :],
                    fin[:].rearrange("p (h w) -> p h w", w=W))

        # batches 0,1 conv on DVE; batches 2,3 conv on PE (concurrent paths)
        y1a, y2a = conv_pair(0, on_pe=False)
        y1b, y2b = conv_pair(1, on_pe=True)
        rest_phase(0, y1a[0], y2a[0])
        rest_phase(1, y1a[1], y2a[1])
        rest_phase(2, y1b[0], y2b[0])
        rest_phase(3, y1b[1], y2b[1])

    nc.compile()
    return nc


_NC_CACHE = None


def _get_nc():
    global _NC_CACHE
    if _NC_CACHE is None:
        _NC_CACHE = build_nc()
    return _NC_CACHE


def _prep_host(inputs):
    """Fold BN into pointwise weights; fold v-bias into final bias."""
    f32 = np.float32
    bf16 = ml_dtypes.bfloat16
    inv_q = (inputs['q_gamma'] / np.sqrt(inputs['q_var'] + EPS)).astype(f32)
    sh_q = (inputs['q_beta'] - inputs['q_mean'] * inv_q).astype(f32)
    A_q = (inputs['q_pw'] * inv_q[None, :]).astype(f32)
    b_q = (inputs['q_pw'].astype(f32) @ sh_q).astype(f32)

    inv_kv = (inputs['kv_gamma'] / np.sqrt(inputs['kv_var'] + EPS)).astype(f32)
    sh_kv = (inputs['kv_beta'] - inputs['kv_mean'] * inv_kv).astype(f32)
    A_kv = (inputs['kv_pw'] * inv_kv[None, :]).astype(f32)
    b_kv = (inputs['kv_pw'].astype(f32) @ sh_kv).astype(f32)
    A_k, A_v = A_kv[:INNER], A_kv[INNER:]
    b_k, b_v = b_kv[:INNER], b_kv[INNER:]

    W2 = inputs['out_w'].astype(f32)
    b2 = (inputs['out_b'].astype(f32) + W2 @ b_v).astype(f32)

    def diag_blocks(taps):
        out = np.zeros((9 * C, 128), f32)
        for ti, (dy, dx) in enumerate(ORDER):
            t = dy * 3 + dx
            for kc_ in range(KC):
                blk = np.diag(taps[kc_ * 128:(kc_ + 1) * 128, t])
                out[ti * C + kc_ * 128:ti * C + (kc_ + 1) * 128, :] = blk
        return out

    qdiag = diag_blocks(inputs['q_dw'].reshape(C, 9).astype(f32))
    kvdiag = diag_blocks(inputs['kv_dw'].reshape(C, 9).astype(f32))

    return {
        'qdiag': qdiag.astype(bf16),
        'kvdiag': kvdiag.astype(bf16),
        'aq': np.ascontiguousarray(A_q.T).astype(bf16),
        'ak': np.ascontiguousarray(A_k.T).astype(bf16),
        'av': np.ascontiguousarray(A_v.T).astype(bf16),
        'w2': np.ascontiguousarray(W2.T).astype(bf16),
        'qtap': np.ascontiguousarray(inputs['q_dw'].reshape(C, 9)).astype(f32),
        'kvtap': np.ascontiguousarray(inputs['kv_dw'].reshape(C, 9)).astype(f32),
        'bq': b_q.reshape(INNER, 1),
        'bk': b_k.reshape(INNER, 1),
        'b2': b2.reshape(C, 1),
    }


def kernel(**inputs):
    inputs = {k: np.asarray(v) for k, v in inputs.items()}
    nc = _get_nc()
    wmap = _prep_host(inputs)
    xb = inputs['x'].astype(ml_dtypes.bfloat16)
    in_maps = []
    for c in range(N_CORES):
        m = dict(wmap)
        m['x'] = np.ascontiguousarray(xb[c * B_LOC:(c + 1) * B_LOC])
        in_maps.append(m)
    res = run_bass_kernel_spmd(nc, in_maps, core_ids=list(range(N_CORES)))
    shards = [res.results[i]['out'] for i in range(N_CORES)]
    return np.concatenate(shards, axis=0).astype(np.float32)
